# revision 49
# baseline (speedup 1.0000x reference)
"""Conditional NT-Xent loss kernel V5 for Trainium2 (8 NeuronCores, SPMD).

Per chunk of 2 consecutive rows from each of zjs/zis (a,b = zjs rows; c,d = zis
rows): need 4 squared norms + 6 pairwise dots, cos_xy = s_xy/(|x||y|),
logits = 2*cos, per-row loss = lse(3 logits) - pos, total = sum / B.

V5 redesign vs V4:
- Host stacks [zjs; zis] into one dram tensor so each load span is a single
  SWDGE dispatch (994ns fixed cost each on Pool) instead of two; 6 spans
  total, ~6us of Pool freed.
- Elementwise volume cut from 14 to 13 col-units per chunk: only {b,c,d} are
  copied PSUM->SBUF (minimum vertex cover of the 6 product pairs); the three
  `a` products read `a` straight from PSUM via a broadcast view (one PSUM
  operand is legal), and all 4 squares come from ONE fused ACT Square over the
  whole transposed set.
- The `b` copy rides the idle DMA engines (HWDGE PSUM->SBUF, no cast needed)
  instead of a compute engine.
- (bc,cd) merge into one DVE mul via overlapping shifted views of TC=[b,c,d];
  remainder groups + bd go to Pool as scalar_tensor_tensor (1.39ns/elem vs
  1.98 for gpsimd tensor_tensor).
- Sets of 3 groups amortize per-instruction init costs; PSUM = 2x3-bank
  transpose tiles + 2 stats banks. Stats matmuls accumulate directly into
  per-piece PSUM tiles read in place by the two pipelined epilogue pieces
  (no drain copies); piece 0 (17 groups) overlaps sets 6-10, only piece 1
  is tail.

Sharding: batch(chunk)-parallel across 8 cores; each core computes two partial
sums [128,1]; host sums partials and divides by B.
"""

import numpy as np

import concourse.bass as bass
import concourse.tile as tile
from concourse import bacc, masks, mybir
from concourse.bass_utils import run_bass_kernel_spmd

N_CORES = 8
B_FULL = 65536            # total rows in zis (== zjs)
ROWS = B_FULL // N_CORES  # 8192 rows per core shard
D = 256
GROUPS = 32               # groups of 128 chunks per core
HW = GROUPS * 512         # half-width of the stacked natural tile (bf16 cols)
F32 = mybir.dt.float32
BF16 = mybir.dt.bfloat16
ALU = mybir.AluOpType
ACTF = mybir.ActivationFunctionType

# set boundaries (group ranges); first set small so compute starts early
SETS = [(0, 2), (2, 5), (5, 8), (8, 11), (11, 14), (14, 17),
        (17, 20), (20, 23), (23, 26), (26, 29), (29, 32)]
# epilogue pieces: (ga, gb, emit_after_set_idx, out col); pieces 0 and 2
# share one PSUM stats bank (program order makes the WAR direction safe)
PIECES = [(0, 17, 5, 0), (17, 29, 9, 1), (29, 32, 10, 2)]
# HBM load spans (groups); one SWDGE dispatch each covers both halves
SPANS = [(0, 1), (1, 3), (3, 6), (6, 11), (11, 17), (17, 24), (24, 32)]

# stats col layout per group: t 0=na 1=nb 2=nc 3=nd 4=ab 5=cd 6=ac 7=bd 8=ad 9=bc


def _epilogue(tc, nc, epi, SP, out, ga, gb, col):
    """Per-chunk softmax math on PSUM stats SP [128, (gb-ga)*10] -> out col."""
    G = gb - ga
    Sv = SP.rearrange("p (g t) -> p g t", t=10)
    norms = Sv[:, :, 0:4]
    svals = Sv[:, :, 4:10]

    LN = epi.tile([128, G * 4], F32, tag=f"ln{ga}")
    LNv = LN[:].rearrange("p (g t) -> p g t", t=4)
    nc.scalar.activation(LNv, norms, ACTF.Ln)

    # q_xy = ln nx + ln ny (pair order ab cd ac bd ad bc)
    Q = epi.tile([128, G * 6], F32, tag=f"q{ga}")
    Qv = Q[:].rearrange("p (g t) -> p g t", t=6)
    # merged pair sums: t0,t1=(0+1, 2+3); t2,t3=(0+2, 1+3); t4,t5=(0+3, 1-2rev)
    LN4 = LN[:].rearrange("p (g t) -> p g t", t=4)
    nc.vector.tensor_add(Qv[:, :, 0:2], LN4[:, :, 0::2], LN4[:, :, 1::2])
    nc.vector.tensor_add(Qv[:, :, 2:4], LN4[:, :, 0:2], LN4[:, :, 2:4])
    nc.vector.tensor_add(Qv[:, :, 4:6], LN4[:, :, 0:2], LN4[:, :, 3:1:-1])

    # rr_xy = exp(-0.5 q) = 1/(|x||y|)
    RQ = epi.tile([128, G * 6], F32, tag=f"rq{ga}")
    RQv = RQ[:].rearrange("p (g t) -> p g t", t=6)
    nc.scalar.activation(RQv, Qv, ACTF.Exp, scale=-0.5)

    # cos_xy = s_xy * rr_xy  (svals is the one PSUM operand)
    C = epi.tile([128, G * 6], F32, tag=f"cos{ga}")
    Cv = C[:].rearrange("p (g t) -> p g t", t=6)
    nc.vector.tensor_mul(Cv, svals, RQv)

    # E_xy = exp(2 cos)
    E = epi.tile([128, G * 6], F32, tag=f"e{ga}")
    Ev = E[:].rearrange("p (g t) -> p g t", t=6)
    nc.scalar.activation(Ev, Cv, ACTF.Exp, scale=2.0)

    # softmax denominators for the 4 rows of each chunk
    DEN = epi.tile([128, G * 4], F32, tag=f"den{ga}")
    DENv = DEN[:].rearrange("p (g t) -> p g t", t=4)
    TMP = epi.tile([128, G * 4], F32, tag=f"tmp{ga}")
    TMPv = TMP[:].rearrange("p (g t) -> p g t", t=4)
    # D0=(E0+E4)+E2, D2=(E5+E1)+E2 ; D1=(E0+E5)+E3, D3=(E4+E1)+E3
    nc.vector.tensor_add(TMPv[:, :, 0:2], Ev[:, :, 0::5], Ev[:, :, 4:0:-3])
    nc.vector.tensor_add(TMPv[:, :, 2:4], Ev[:, :, 0::4], Ev[:, :, 5:0:-4])
    nc.vector.tensor_add(
        DENv[:, :, 0:2], TMPv[:, :, 0:2], Ev[:, :, 2:3].broadcast_to([128, G, 2])
    )
    nc.vector.tensor_add(
        DENv[:, :, 2:4], TMPv[:, :, 2:4], Ev[:, :, 3:4].broadcast_to([128, G, 2])
    )

    LD = epi.tile([128, G * 4], F32, tag=f"ld{ga}")
    LDv = LD[:].rearrange("p (g t) -> p g t", t=4)
    nc.scalar.activation(LDv, DENv, ACTF.Ln)

    LG = epi.tile([128, G], F32, tag=f"lg{ga}")
    nc.vector.reduce_sum(
        LG[:].rearrange("p (g o) -> p g o", o=1), LDv, axis=mybir.AxisListType.X
    )

    T1 = epi.tile([128, G], F32, tag=f"t1{ga}")
    nc.vector.tensor_add(
        T1[:].rearrange("p (g o) -> p g o", o=1), Cv[:, :, 2:3], Cv[:, :, 3:4]
    )

    # loss per chunk-col = LG - 4*T1
    LC = epi.tile([128, G], F32, tag=f"lc{ga}")
    nc.vector.scalar_tensor_tensor(
        out=LC[:], in0=T1[:], scalar=-4.0, in1=LG[:], op0=ALU.mult, op1=ALU.add
    )

    ACC = epi.tile([128, 1], F32, tag=f"acc{ga}")
    nc.vector.reduce_sum(ACC[:], LC[:], axis=mybir.AxisListType.X)
    nc.sync.dma_start(out=out[:, col : col + 1], in_=ACC[:])


def _trace_kernel(tc, nc, z, out):
    # host pre-interleaves shards as [g, p, half, two, f] so a span load is a
    # 3-dim AP with (half two f) = 4KB contiguous runs per (g, p)
    z_v = z.rearrange("(g p h two) f -> p g (h two f)", g=GROUPS, p=128, two=2)

    with (
        tc.tile_pool(name="consts", bufs=1) as consts,
        tc.tile_pool(name="nat", bufs=1) as nat,
        tc.tile_pool(name="tp", bufs=2, space="PSUM") as tp,
        tc.tile_pool(name="red", bufs=1, space="PSUM") as red,
        tc.tile_pool(name="tcp", bufs=6) as tcp,
        tc.tile_pool(name="sqp", bufs=6) as sqp,
        tc.tile_pool(name="mdp", bufs=6) as mdp,
        tc.tile_pool(name="mbp", bufs=6) as mbp,
        tc.tile_pool(name="epi", bufs=1) as epi,
    ):
        ident = consts.tile([128, 128], BF16)
        masks.make_identity(nc, ident[:])
        ones = consts.tile([128, 1], BF16)
        nc.gpsimd.memset(ones[:], 1.0)

        # natural tile, group-major interleaved: per group 1024 cols
        # [J two=0 (a), J two=1 (b), I two=0 (c), I two=1 (d)] x 256 f.
        # Early spans dispatched upfront; later ones interleave into the set
        # loop so Pool's ~1.1us/dispatch doesn't monopolize its early queue.
        ZB = nat.tile([128, 2 * HW], BF16)

        def emit_load(d):
            g0, g1 = SPANS[d]
            nc.gpsimd.dma_start(
                out=ZB[:, g0 * 1024 : g1 * 1024].rearrange(
                    "p (g c) -> p g c", g=g1 - g0
                ),
                in_=z_v[:, g0:g1, :],
            )

        for d in range(4):
            emit_load(d)

        # per-piece PSUM stats views (epilogue reads them in place);
        # pieces 0 and 2 share one bank-granular tile
        sp_a = red.tile([128, 200], F32, tag="sp_a", name="sp_a")
        sp_b = red.tile([128, 120], F32, tag="sp_b", name="sp_b")
        SPs = {0: sp_a[:, 0:170], 17: sp_b[:], 29: sp_a[:, 170:200]}

        def emit_transposes(si):
            g0, g1 = SETS[si]
            G = g1 - g0
            # PT group-major layout: per group [a(256) b(256) c(256) d(256)]
            PT = tp.tile([128, G * 1024], BF16, tag="pt")
            if si == 0:
                # PE pstate warm-up: dummy transposes keep PE continuously
                # busy from t~0.8us so the ramp reaches full speed
                for _ in range(16):
                    nc.tensor.transpose(PT[:, 0:128], ident[:], ident[:])
            for g in range(G):
                base = g * 1024
                nbase = (g0 + g) * 1024
                for k in range(8):
                    nc.tensor.transpose(
                        PT[:, base + 128 * k : base + 128 * (k + 1)],
                        ZB[:, nbase + 128 * k : nbase + 128 * (k + 1)],
                        ident[:],
                    )
            return PT

        def emit_body(si, PT):
            g0, g1 = SETS[si]
            G = g1 - g0
            PTv = PT[:].rearrange("p (g r f) -> p g r f", r=4, f=256)

            # squares + SBUF copies of {b,c,d} (minimum vertex cover of the
            # 6 product pairs). Same-region PSUM readers serialize in emission
            # order, so set 0 avoids the ACT->DVE chain entirely: DVE copies
            # first and squares b,c,d itself from the copy; ACT squares only
            # `a`. Steady-state sets use one fused ACT Square (ACT has slack).
            TC = tcp.tile([128, G * 768], BF16, tag="tc")
            TCv = TC[:].rearrange("p (g t f) -> p g t f", t=3, f=256)
            SQ = sqp.tile([128, G * 1024], BF16, tag="sq")
            SQv = SQ[:].rearrange("p (g r f) -> p g r f", r=4, f=256)
            nc.scalar.activation(SQ[:], PT[:], ACTF.Square)
            nc.vector.tensor_copy(TCv, PTv[:, :, 1:4, :])

            # products (ab, ac, ad): `a` is the single PSUM operand, broadcast
            MD = mdp.tile([128, G * 768], BF16, tag="md")
            MDv = MD[:].rearrange("p (g t f) -> p g t f", t=3, f=256)
            nc.vector.tensor_mul(
                MDv, TCv, PTv[:, :, 0:1, :].broadcast_to([128, G, 3, 256])
            )

            # products (bc, bd): (c,d) * broadcast b, all SBUF. Pool takes
            # the first G-1 groups plus (on even sets) bc of the last group;
            # DVE takes the remainder (Pool runs at 1.98 vs 0.52 ns/elem).
            MB = mbp.tile([128, G * 512], BF16, tag="mb")
            MBv = MB[:].rearrange("p (g t f) -> p g t f", t=2, f=256)
            ks = G - 1
            nc.gpsimd.tensor_mul(
                MBv[:, 0:ks],
                TCv[:, 0:ks, 1:3, :],
                TCv[:, 0:ks, 0:1, :].broadcast_to([128, ks, 2, 256]),
            )
            nc.vector.tensor_mul(
                MBv[:, ks:G],
                TCv[:, ks:G, 1:3, :],
                TCv[:, ks:G, 0:1, :].broadcast_to([128, G - ks, 2, 256]),
            )
            # product (cd) on DVE
            MF = mbp.tile([128, G * 256], BF16, tag="mf")
            MFv = MF[:].rearrange("p (g f) -> p g f", f=256)
            nc.vector.tensor_mul(MFv, TCv[:, :, 1, :], TCv[:, :, 2, :])

            # chunk-major reductions (PE ones-matmuls) into the piece's
            # PSUM stats tile. Stats sourced from the Pool-made MB tile are
            # DEFERRED one set so PE's in-order queue never waits on Pool.
            pa = max(p[0] for p in PIECES if p[0] <= g0)
            SP = SPs[pa]
            mb_mms = []
            for g in range(G):
                cols = {
                    0: (SQ, g * 1024),            # na
                    1: (SQ, g * 1024 + 256),      # nb
                    2: (SQ, g * 1024 + 512),      # nc
                    3: (SQ, g * 1024 + 768),      # nd
                    4: (MD, g * 768),             # ab
                    5: (MF, g * 256),             # cd
                    6: (MD, g * 768 + 256),       # ac
                    8: (MD, g * 768 + 512),       # ad
                }
                # DVE-sourced stats first so PE's in-order queue isn't gated
                # on the (later-emitted) ACT squares
                for t in (0, 1, 2, 3, 4, 5, 6, 8):
                    src, off = cols[t]
                    emit_mm(SP, (g0 + g - pa) * 10 + t, src, off)
                for t in (7, 9):  # bd, bc from MB
                    off = g * 512 + (256 if t == 7 else 0)
                    mb_mms.append((SP, (g0 + g - pa) * 10 + t, MB, off))
            pending.append(mb_mms)

        def emit_mm(SP, col, src, off):
            for sh in range(2):
                nc.tensor.matmul(
                    SP[:, col : col + 1],
                    src[:, off + sh * 128 : off + (sh + 1) * 128],
                    ones[:, 0:1],
                    start=(sh == 0),
                    stop=(sh == 1),
                )

        # software pipeline: transposes one set ahead of each set's body;
        # Pool-dependent matmuls of set si are flushed TWO sets later so PE's
        # in-order queue never waits on Pool's slower muls
        pending = []
        dispatch_at = {1: 4, 3: 5, 5: 6}
        PTq = [emit_transposes(0)]
        for si in range(len(SETS)):
            if si + 1 < len(SETS):
                PTq.append(emit_transposes(si + 1))
            if len(pending) >= 2:
                for args in pending.pop(0):
                    emit_mm(*args)
            emit_body(si, PTq[si])
            if si in dispatch_at:
                emit_load(dispatch_at[si])
            for ga, gb, after, col in PIECES:
                if si == after:
                    while pending:
                        for args in pending.pop(0):
                            emit_mm(*args)
                    _epilogue(tc, nc, epi, SPs[ga], out, ga, gb, col)


def _merge_act_table_loads(nc):
    """Collapse greedy per-func act-table loads into one load of the set
    that contains every function this kernel uses (square/ln/exp)."""
    from concourse.hw_specs import get_activation_tables

    try:
        tables = get_activation_tables(nc.m.arch)
    except Exception:
        return
    target = None
    need = {
        ACTF.Square,
        ACTF.Ln,
        ACTF.Exp,
        ACTF.Identity,
    }
    for idx, (name, funcs) in enumerate(tables.items()):
        if need.issubset(funcs):
            target = idx
            break
    if target is None:
        return
    for blk in nc.m.functions[0].blocks:
        loads = [
            inst
            for inst in blk.instructions
            if isinstance(inst, mybir.InstLoadActFuncSet)
        ]
        if not loads:
            continue
        # only safe to drop loads that carry no semaphore waits/updates
        def _sync_free(inst):
            si = inst.sync_info
            return si is None or (len(si.on_wait) == 0 and len(si.on_update) == 0)

        if not all(_sync_free(l) for l in loads[1:]):
            for l in loads:
                l.act_func_set_id = target
            continue
        loads[0].act_func_set_id = target
        keep = set(id(l) for l in loads[1:])
        insts = [i for i in blk.instructions if id(i) not in keep]
        del blk.instructions[:]
        for i in insts:
            blk.instructions.append(i)


_NC_CACHE = None


def _build_nc():
    global _NC_CACHE
    if _NC_CACHE is not None:
        return _NC_CACHE
    nc = bacc.Bacc("TRN2", target_bir_lowering=False, debug=False, num_devices=N_CORES)
    z = nc.dram_tensor("z", [2 * ROWS, D], F32, kind="ExternalInput")
    out = nc.dram_tensor("out", [128, 3], F32, kind="ExternalOutput")
    with tile.TileContext(nc) as tc:
        _trace_kernel(tc, nc, z.ap(), out.ap())
    nc.compile()
    _merge_act_table_loads(nc)
    _NC_CACHE = nc
    return nc


def run_cores(zis, zjs, trace=False):
    """Run the SPMD kernel; returns (list of per-core out arrays, results)."""
    nc = _build_nc()
    zis = np.ascontiguousarray(np.asarray(zis, dtype=np.float32))
    zjs = np.ascontiguousarray(np.asarray(zjs, dtype=np.float32))
    in_maps = []
    for i in range(N_CORES):
        sl = slice(i * ROWS, (i + 1) * ROWS)
        zj = zjs[sl].reshape(GROUPS, 128, 2, D)
        zi = zis[sl].reshape(GROUPS, 128, 2, D)
        z = np.stack([zj, zi], axis=2).reshape(2 * ROWS, D)
        in_maps.append({"z": np.ascontiguousarray(z)})
    res = run_bass_kernel_spmd(nc, in_maps, list(range(N_CORES)), trace=trace)
    return [r["out"] for r in res.results], res


def kernel(zis, zjs):
    outs, _ = run_cores(zis, zjs, trace=False)
    total = np.sum([o.astype(np.float64).sum() for o in outs])
    return np.asarray(total / B_FULL, dtype=np.float32)
